# revision 8
# baseline (speedup 1.0000x reference)
"""CrossAttention3D kernel for Trainium2 (Bass/Tile), SPMD over 8 NeuronCores.

Problem (full shapes): q_inputs [4,4096,128], kv_inputs [4,4096,128],
Wq/Wk/Wv [128,128], bq/bk/bv [128].
    q = q_in @ Wq + bq ; k = kv_in @ Wk + bk ; v = kv_in @ Wv + bv
    out = softmax(q k^T / sqrt(128)) @ v

Sharding: data-parallel over batch (4) x query-sequence halves (2) = 8 shards.
Each core: xq [2048,128] (query slice), xkv [4096,128] (its batch's full KV).

Numerics: QK^T in float32r (TF32-like); exp output, V tiles and the
denominator accumulation in float16.  exp is computed as exp(s/sqrt(F) - 2)
(the shift cancels in softmax) so fp16 denominators stay in [~700, ~2000],
far from overflow; measured end-to-end rel err ~2.5e-4.

Structure (per core):
  - Inputs DMA'd as [128, 512] tiles via the row-interleaved view
    (g p t) c -> g p (t c), split across the sync and scalar DMA queues.
  - TensorE transposes put C on partitions; projections in f32r:
    kT=[F,Nkv], qT=[F,Nq] (+bias via tensor_scalar eviction), vT -> re-
    transposed into fp16 vt tiles [m,128f] for the PV matmul.
  - Attention per 1024-wide query chunk, per kv tile mi (lag-1 PV):
      sT = kT[:,mi]^T qT[:,chunk]   2x 512-wide f32r matmuls -> PSUM
      E  = exp(scale*sT - 2)        one ScalarE ACTIVATE -> e (fp16)
      outT += vt[mi]^T E            2x fp16 matmuls, PSUM accumulate
      acc += E                      fp16 TT add on DVE (2x SIMD mode)
    oT PSUM is double-buffered across chunks; QK PSUM pool doubles as the
    scratch bank for transposes/projections (8 PSUM banks exactly).
  - Chunk tail: ones16^T acc matmul folds partitions -> dn[1,:] (the last
    E tile is folded directly, skipping its TT add); DVE reciprocal from
    PSUM; 8 tiny 1-partition matmuls transpose 1/d into per-column scalars
    dcol[128,1]; output transposes run on the *unnormalized* oT eviction in
    parallel, and normalization is fused into the PSUM eviction as
    tensor_scalar_mul with dcol - no partition_broadcast, no [128,N]
    reciprocal/multiply.
  - Chunk 0's tail is emitted *after* chunk 1's first 4 kv tiles so the
    in-order PE queue never stalls on the tail's dependencies.
"""

import math
from contextlib import ExitStack

import numpy as np

P = 128
B_FULL, NQ_FULL, NKV, C, F = 4, 4096, 4096, 128, 128
N_CORES = 8
NQ = B_FULL * NQ_FULL // N_CORES  # 2048 queries per core
SCALE = 1.0 / math.sqrt(F)
EXP_BIAS = -2.0  # exp(s - 2): cancels in softmax, keeps fp16 sums small

NKV_T = NKV // P  # 32 kv tiles
TQ = 4  # row interleave factor (512-row groups)
NGK = NKV // (P * TQ)  # 8 kv groups
NCHUNK = 1024
NCH = NQ // NCHUNK  # 2 chunks
MM = 512  # max moving free dim

_CACHE = {}


def _build_nc():
    import concourse.bacc as bacc
    import concourse.tile as tile
    from concourse import mybir
    from concourse.masks import make_identity

    FP32 = mybir.dt.float32
    F32R = mybir.dt.float32r
    FP16 = mybir.dt.float16
    ADD = mybir.AluOpType.add
    EXP = mybir.ActivationFunctionType.Exp

    nc = bacc.Bacc("TRN2", target_bir_lowering=False, debug=False)

    xq = nc.dram_tensor("xq", [NQ, C], FP32, kind="ExternalInput")
    xkv = nc.dram_tensor("xkv", [NKV, C], FP32, kind="ExternalInput")
    wq = nc.dram_tensor("wq", [C, F], FP32, kind="ExternalInput")
    wk = nc.dram_tensor("wk", [C, F], FP32, kind="ExternalInput")
    wv = nc.dram_tensor("wv", [C, F], FP32, kind="ExternalInput")
    bq = nc.dram_tensor("bq", [F, 1], FP32, kind="ExternalInput")
    bk = nc.dram_tensor("bk", [F, 1], FP32, kind="ExternalInput")
    bv = nc.dram_tensor("bv", [F, 1], FP32, kind="ExternalInput")
    out = nc.dram_tensor("out", [NQ, F], FP32, kind="ExternalOutput")

    xq_v = xq.rearrange("(g p t) c -> g p (t c)", p=P, t=TQ)
    xkv_v = xkv.rearrange("(g p t) c -> g p (t c)", p=P, t=TQ)
    out_v = out.rearrange("(g p t) c -> g p t c", p=P, t=TQ)

    with tile.TileContext(nc) as tc, ExitStack() as ctx:
        const = ctx.enter_context(tc.tile_pool(name="const", bufs=1))

        xpool = ctx.enter_context(tc.tile_pool(name="xpool", bufs=4))
        spsum = ctx.enter_context(tc.tile_pool(name="spsum", bufs=2, space="PSUM"))
        opsum = ctx.enter_context(tc.tile_pool(name="opsum", bufs=2, space="PSUM"))
        epool = ctx.enter_context(tc.tile_pool(name="epool", bufs=6))
        apool = ctx.enter_context(tc.tile_pool(name="apool", bufs=2))
        npool = ctx.enter_context(tc.tile_pool(name="npool", bufs=4))
        otpool = ctx.enter_context(tc.tile_pool(name="otpool", bufs=4))

        # ---- input DMAs first (split across the two hardware DMA queues) --
        xthead = []
        for g in range(2):
            xt = xpool.tile([P, TQ * C], FP32, tag="xt", name=f"xq_{g}")
            (nc.sync if g == 0 else nc.scalar).dma_start(xt, xq_v[g])
            xthead.append(xt)
        xtkv0 = xpool.tile([P, TQ * C], FP32, tag="xt", name="xkv_0")
        nc.scalar.dma_start(xtkv0, xkv_v[0])
        _PRELOADED = {("q", 0): xthead[0], ("q", 1): xthead[1], ("k", 0): xtkv0}

        identity = const.tile([P, P], FP32)
        make_identity(nc, identity)
        identity_r = const.tile([P, P], F32R)
        nc.vector.tensor_copy(identity_r, identity)

        w_s = {}
        for i, (name, drt) in enumerate((("wq", wq), ("wk", wk), ("wv", wv))):
            raw = const.tile([C, F], FP32, name=f"{name}_raw")
            (nc.sync if i % 2 == 0 else nc.scalar).dma_start(raw, drt[:])
            rs = const.tile([C, F], F32R, name=f"{name}_s")
            nc.vector.tensor_copy(rs, raw)
            w_s[name] = rs
        bq_s = const.tile([F, 1], FP32)
        nc.scalar.dma_start(bq_s, bq[:])
        bk_s = const.tile([F, 1], FP32)
        nc.sync.dma_start(bk_s, bk[:])
        bv_s = const.tile([F, 1], FP32)
        nc.scalar.dma_start(bv_s, bv[:])

        ebias = const.tile([P, 1], FP32)
        nc.vector.memset(ebias, EXP_BIAS)
        ones_f = const.tile([P, 1], FP32)
        nc.vector.memset(ones_f, 1.0)
        ones16 = const.tile([P, 1], FP16)
        nc.vector.tensor_copy(ones16, ones_f)
        one1f = const.tile([1, 1], FP32)
        nc.vector.memset(one1f, 1.0)

        kvT = const.tile([P, NKV], F32R)  # [c, m]
        qTin = const.tile([P, NQ], F32R)  # [c, n]
        kT = const.tile([P, NKV], F32R)  # [f, m]
        qT = const.tile([P, NQ], F32R)  # [f, n]
        vT = const.tile([P, NKV], F32R)  # [f, m]
        vt = const.tile([P, NKV_T, F], FP16)  # [m%128, m//128, f]

        def load_group(view, g, dstT, tagc, evict_engine):
            """DMA one [128, 512] interleaved group, transpose its 4 blocks
            into one PSUM tile, evict coalesced (casts to f32r)."""
            xt = _PRELOADED.get((tagc, g))
            if xt is None:
                xt = xpool.tile([P, TQ * C], FP32, tag="xt", name=f"x{tagc}_{g}")
                (nc.sync if g % 2 == 0 else nc.scalar).dma_start(xt, view[g])
            pt = spsum.tile([P, TQ * P], FP32, tag="sp", name=f"p{tagc}_{g}")
            for t in range(TQ):
                nc.tensor.transpose(
                    pt[:, t * P : (t + 1) * P], xt[:, t * P : (t + 1) * P], identity
                )
            col = g * (P * TQ)
            if evict_engine == "act":
                nc.scalar.copy(dstT[:, col : col + TQ * P], pt)
            else:
                nc.vector.tensor_copy(dstT[:, col : col + TQ * P], pt)

        def project_slice(wname, srcT, dstT, bias, j):
            pp = spsum.tile([P, MM], FP32, tag="sp", name=f"pj{wname}_{j}")
            nc.tensor.matmul(
                pp, w_s[wname], srcT[:, j * MM : (j + 1) * MM], start=True, stop=True
            )
            nc.vector.tensor_scalar_add(dstT[:, j * MM : (j + 1) * MM], pp, bias)

        def vt_group(g):
            """Transpose 4 vT blocks into fp16 vt tiles (one coalesced evict)."""
            pv = spsum.tile([P, TQ * P], F32R, tag="sp", name=f"pvt_{g}")
            for t in range(TQ):
                i = g * TQ + t
                nc.tensor.transpose(
                    pv[:, t * P : (t + 1) * P],
                    vT[:, i * P : (i + 1) * P],
                    identity_r,
                )
            nc.vector.tensor_copy(vt[:, g * TQ : (g + 1) * TQ, :], pv)

        # ---- queries for chunk 0 (rest interleaved below) ----
        for g in range(2):
            load_group(xq_v, g, qTin, "q", "dve")
        for j in range(2):
            project_slice("wq", qTin, qT, bq_s, j)

        # ---- attention chunk emitter (lag-1 PV, fp16 denominator) ----
        chunk_state = {}

        def attn_start(nch):
            oT = opsum.tile([P, NCHUNK], FP32, tag="oT", name=f"oT_{nch}")
            acc = apool.tile([P, NCHUNK], FP16, tag="acc", name=f"acc_{nch}")
            chunk_state[nch] = dict(oT=oT, acc=acc, e0=None, prev=None)

        def emit_pv(nch, e, mi):
            st = chunk_state[nch]
            for h in range(NCHUNK // MM):
                nc.tensor.matmul(
                    st["oT"][:, h * MM : (h + 1) * MM],
                    vt[:, mi, :],
                    e[:, h * MM : (h + 1) * MM],
                    start=(mi == 0),
                    stop=(mi == NKV_T - 1),
                )
            # fp16 denominator partial sums on DVE (2x SIMD mode).  The last
            # tile skips its add: attn_finish folds it straight into dn.
            if mi == 0:
                st["e0"] = e
            elif mi == 1:
                nc.vector.tensor_tensor(st["acc"], st["e0"], e, ADD)
            elif mi < NKV_T - 1:
                nc.vector.tensor_tensor(st["acc"], st["acc"], e, ADD)

        def attn_mi(nch, mi):
            st = chunk_state[nch]
            nq0 = nch * NCHUNK
            sp = spsum.tile([P, NCHUNK], FP32, tag="sp", name=f"sp_{nch}_{mi}")
            for h in range(NCHUNK // MM):
                nc.tensor.matmul(
                    sp[:, h * MM : (h + 1) * MM],
                    kT[:, mi * P : (mi + 1) * P],
                    qT[:, nq0 + h * MM : nq0 + (h + 1) * MM],
                    start=True,
                    stop=True,
                )
            e = epool.tile([P, NCHUNK], FP16, tag="e", name=f"e_{nch}_{mi}")
            nc.scalar.activation(e, sp, EXP, bias=ebias, scale=SCALE)
            if st["prev"] is not None:
                emit_pv(nch, *st["prev"])
            st["prev"] = (e, mi)

        def attn_finish(nch):
            st = chunk_state[nch]
            e_last, _ = st["prev"]
            emit_pv(nch, *st["prev"])
            # partition-fold: dn[1, n] = sum_m (acc + e_last)[m, n]
            dn = spsum.tile([1, NCHUNK], FP32, tag="sp", name=f"dn_{nch}")
            for h in range(NCHUNK // MM):
                hs = slice(h * MM, (h + 1) * MM)
                nc.tensor.matmul(dn[:, hs], ones16, st["acc"][:, hs], start=True, stop=False)
                nc.tensor.matmul(dn[:, hs], ones16, e_last[:, hs], start=False, stop=True)
            rdn = npool.tile([1, NCHUNK], FP32, tag="rdn", name=f"rdn_{nch}")
            nc.vector.reciprocal_approx_fast(rdn, dn)
            # transpose 1/d rows into per-column scalars dcol[128, j]
            dcol = spsum.tile([P, NCHUNK // P], FP32, tag="sp", name=f"dcol_{nch}")
            for j in range(NCHUNK // P):
                nc.tensor.matmul(
                    dcol[:, j : j + 1],
                    rdn[0:1, j * P : (j + 1) * P],
                    one1f,
                    start=True,
                    stop=True,
                )
            dcol_sb = npool.tile([P, NCHUNK // P], FP32, tag="dcol", name=f"dcs_{nch}")
            nc.vector.tensor_copy(dcol_sb, dcol)
            # unnormalized eviction of oT (runs in parallel with the dn chain)
            osb = npool.tile([P, NCHUNK], FP32, tag="osb", name=f"osb_{nch}")
            nc.scalar.copy(osb, st["oT"])
            for gg in range(NCHUNK // (P * TQ)):
                g = nch * (NCHUNK // (P * TQ)) + gg
                tp = spsum.tile([P, TQ * P], FP32, tag="sp", name=f"tp_{nch}_{gg}")
                for t in range(TQ):
                    j = gg * TQ + t
                    nc.tensor.transpose(
                        tp[:, t * P : (t + 1) * P], osb[:, j * P : (j + 1) * P], identity
                    )
                ot = otpool.tile([P, TQ * P], FP32, tag="ot", name=f"ot_{nch}_{gg}")
                for t in range(TQ):
                    j = gg * TQ + t
                    nc.vector.tensor_scalar_mul(
                        ot[:, t * P : (t + 1) * P],
                        tp[:, t * P : (t + 1) * P],
                        dcol_sb[:, j : j + 1],
                    )
                (nc.sync if gg % 2 == 0 else nc.scalar).dma_start(
                    out_v[g], ot.rearrange("p (t c) -> p t c", t=TQ)
                )

        # ---- interleave kv-group loading/projection with chunk-0 attention --
        attn_start(0)
        for g in range(NGK):
            load_group(xkv_v, g, kvT, "k", "act")
            project_slice("wk", kvT, kT, bk_s, g)
            project_slice("wv", kvT, vT, bv_s, g)
            vt_group(g)
            if g < 2:  # finish the q-side for chunk 1
                load_group(xq_v, g + 2, qTin, "q", "dve")
                project_slice("wq", qTin, qT, bq_s, g + 2)
            for t in range(TQ):
                attn_mi(0, g * TQ + t)

        # chunk 1 head first, then chunk-0 tail (keeps the in-order PE queue
        # busy while chunk 0's tail dependencies settle), then the rest
        attn_start(1)
        for mi in range(4):
            attn_mi(1, mi)
        attn_finish(0)
        for mi in range(4, NKV_T):
            attn_mi(1, mi)
        attn_finish(1)

    nc.compile()
    return nc


def _get_nc():
    if "nc" not in _CACHE:
        _CACHE["nc"] = _build_nc()
    return _CACHE["nc"]


def run(inputs, trace=False, **kwargs):
    """Run on 8 cores; returns (full_output [4,4096,128], BassKernelResults)."""
    from concourse.bass_utils import run_bass_kernel_spmd

    q_in = np.ascontiguousarray(np.asarray(inputs["q_inputs"], dtype=np.float32))
    kv_in = np.ascontiguousarray(np.asarray(inputs["kv_inputs"], dtype=np.float32))
    wq = np.ascontiguousarray(np.asarray(inputs["Wq"], dtype=np.float32))
    wk = np.ascontiguousarray(np.asarray(inputs["Wk"], dtype=np.float32))
    wv = np.ascontiguousarray(np.asarray(inputs["Wv"], dtype=np.float32))
    bq = np.ascontiguousarray(np.asarray(inputs["bq"], dtype=np.float32).reshape(F, 1))
    bk = np.ascontiguousarray(np.asarray(inputs["bk"], dtype=np.float32).reshape(F, 1))
    bv = np.ascontiguousarray(np.asarray(inputs["bv"], dtype=np.float32).reshape(F, 1))

    halves = NQ_FULL // NQ  # 2
    in_maps = []
    for core in range(N_CORES):
        b, h = core // halves, core % halves
        in_maps.append(
            {
                "xq": np.ascontiguousarray(q_in[b, h * NQ : (h + 1) * NQ]),
                "xkv": np.ascontiguousarray(kv_in[b]),
                "wq": wq,
                "wk": wk,
                "wv": wv,
                "bq": bq,
                "bk": bk,
                "bv": bv,
            }
        )

    nc = _get_nc()
    res = run_bass_kernel_spmd(
        nc, in_maps, core_ids=list(range(N_CORES)), trace=trace, **kwargs
    )

    full = np.empty((B_FULL, NQ_FULL, F), dtype=np.float32)
    for core in range(N_CORES):
        b, h = core // halves, core % halves
        full[b, h * NQ : (h + 1) * NQ] = res.results[core]["out"]
    return full, res


def kernel(**inputs):
    full, _ = run(inputs, trace=False)
    return full


# revision 13
# speedup vs baseline: 1.3886x; 1.3886x over previous
"""CrossAttention3D kernel for Trainium2 (Bass/Tile), SPMD over 8 NeuronCores.

Problem (full shapes): q_inputs [4,4096,128], kv_inputs [4,4096,128],
Wq/Wk/Wv [128,128], bq/bk/bv [128].
    q = q_in @ Wq + bq ; k = kv_in @ Wk + bk ; v = kv_in @ Wv + bv
    out = softmax(q k^T / sqrt(128)) @ v

Sharding: data-parallel over batch (4) x query-sequence halves (2) = 8 shards.
Each core: xq [2048,128] (query slice), xkv [4096,128] (its batch's full KV).

Numerics: QK^T in float32r (TF32-like); exp output, V tiles and the
denominator accumulation in float16 (denominators stay < ~14k, well under
fp16 max for these inputs; partial sums fold in fp32 PSUM).  Measured
end-to-end rel err ~2.5e-4.

Structure (per core):
  - Inputs DMA'd as [128, 512] tiles via the row-interleaved view
    (g p t) c -> g p (t c), split across the sync and scalar DMA queues.
  - TensorE transposes put C on partitions; f32r projections produce
    kT=[F,Nkv], qT=[F,Nq] (bias via tensor_scalar eviction); vT is
    re-transposed into fp16 vt tiles [m,128f] for the PV matmul.  KV group
    g+1 is loaded/projected *before* group g's attention block so the
    DVE/ScalarE PSUM evictions never sit on the PE's critical path.
  - Attention per 1024-wide query chunk, per kv tile mi:
      sT = kT[:,mi]^T qT[:,chunk]   2x 512-wide f32r matmuls -> PSUM
      E  = exp(scale*sT)            one ScalarE ACTIVATE -> e (fp16)
      outT += vt[mi]^T E            2x fp16 matmuls, PSUM accumulate
      acc += E                      fp16 TT add on DVE (2x SIMD mode)
    PV matmuls lag the QK/exp pipeline (lag 1 in chunk 0, lag 2 in chunk 1
    so the chunk-0 tail and the single-buffered oT drain hide behind
    chunk 1's first QK/exp steps).
  - Chunk tail: ones16^T acc fp16 matmuls fold partitions -> dn[1,:] per
    512-half (the last E tile folds directly, skipping its TT add); DVE
    reciprocal straight from PSUM; 4 tiny f32r matmuls per half transpose
    1/d into per-column scalars dcol[128,1]; output transposes run on the
    *unnormalized* oT eviction in parallel, and normalization fuses into
    the PSUM eviction (tensor_scalar_mul on DVE / activation-scale Copy on
    ScalarE, alternating) - no partition_broadcast, no [128,N] reciprocal.
"""

import math
from contextlib import ExitStack

import numpy as np

P = 128
B_FULL, NQ_FULL, NKV, C, F = 4, 4096, 4096, 128, 128
N_CORES = 8
NQ = B_FULL * NQ_FULL // N_CORES  # 2048 queries per core
SCALE = 1.0 / math.sqrt(F)

NKV_T = NKV // P  # 32 kv tiles
TQ = 4  # row interleave factor (512-row groups)
NGK = NKV // (P * TQ)  # 8 kv groups
NCHUNK = 1024
NCH = NQ // NCHUNK  # 2 chunks
MM = 512  # max moving free dim

_CACHE = {}


def _build_nc():
    import concourse.bacc as bacc
    import concourse.tile as tile
    from concourse import mybir
    from concourse.masks import make_identity

    FP32 = mybir.dt.float32
    F32R = mybir.dt.float32r
    FP16 = mybir.dt.float16
    ADD = mybir.AluOpType.add
    EXP = mybir.ActivationFunctionType.Exp
    CPY = mybir.ActivationFunctionType.Copy

    nc = bacc.Bacc("TRN2", target_bir_lowering=False, debug=False)

    xq = nc.dram_tensor("xq", [NQ, C], FP32, kind="ExternalInput")
    xkv = nc.dram_tensor("xkv", [NKV, C], FP32, kind="ExternalInput")
    wq = nc.dram_tensor("wq", [C, F], FP32, kind="ExternalInput")
    wk = nc.dram_tensor("wk", [C, F], FP32, kind="ExternalInput")
    wv = nc.dram_tensor("wv", [C, F], FP32, kind="ExternalInput")
    bq = nc.dram_tensor("bq", [F, 1], FP32, kind="ExternalInput")
    bk = nc.dram_tensor("bk", [F, 1], FP32, kind="ExternalInput")
    bv = nc.dram_tensor("bv", [F, 1], FP32, kind="ExternalInput")
    out = nc.dram_tensor("out", [NQ, F], FP32, kind="ExternalOutput")

    xq_v = xq.rearrange("(g p t) c -> g p (t c)", p=P, t=TQ)
    xkv_v = xkv.rearrange("(g p t) c -> g p (t c)", p=P, t=TQ)
    out_v = out.rearrange("(g p t) c -> g p t c", p=P, t=TQ)

    with tile.TileContext(nc) as tc, ExitStack() as ctx:
        const = ctx.enter_context(tc.tile_pool(name="const", bufs=1))

        xpool = ctx.enter_context(tc.tile_pool(name="xpool", bufs=4))
        spsum = ctx.enter_context(tc.tile_pool(name="spsum", bufs=2, space="PSUM"))
        opsum = ctx.enter_context(tc.tile_pool(name="opsum", bufs=1, space="PSUM"))
        pwork = ctx.enter_context(tc.tile_pool(name="pwork", bufs=2, space="PSUM"))
        epool = ctx.enter_context(tc.tile_pool(name="epool", bufs=6))
        apool = ctx.enter_context(tc.tile_pool(name="apool", bufs=2))
        npool = ctx.enter_context(tc.tile_pool(name="npool", bufs=6))
        otpool = ctx.enter_context(tc.tile_pool(name="otpool", bufs=4))

        # ---- input DMAs first (split across the two hardware DMA queues) --
        xthead = []
        for g in range(2):
            xt = xpool.tile([P, TQ * C], FP32, tag="xt", name=f"xq_{g}")
            (nc.sync if g == 0 else nc.scalar).dma_start(xt, xq_v[g])
            xthead.append(xt)
        xtkv0 = xpool.tile([P, TQ * C], FP32, tag="xt", name="xkv_0")
        nc.scalar.dma_start(xtkv0, xkv_v[0])
        _PRELOADED = {("q", 0): xthead[0], ("q", 1): xthead[1], ("k", 0): xtkv0}

        identity = const.tile([P, P], FP32)
        make_identity(nc, identity)
        identity_r = const.tile([P, P], F32R)
        nc.vector.tensor_copy(identity_r, identity)

        w_s = {}
        for i, (name, drt) in enumerate((("wq", wq), ("wk", wk), ("wv", wv))):
            raw = const.tile([C, F], FP32, name=f"{name}_raw")
            (nc.sync if i % 2 == 0 else nc.scalar).dma_start(raw, drt[:])
            rs = const.tile([C, F], F32R, name=f"{name}_s")
            nc.vector.tensor_copy(rs, raw)
            w_s[name] = rs
        bq_s = const.tile([F, 1], FP32)
        nc.scalar.dma_start(bq_s, bq[:])
        bk_s = const.tile([F, 1], FP32)
        nc.sync.dma_start(bk_s, bk[:])
        bv_s = const.tile([F, 1], FP32)
        nc.scalar.dma_start(bv_s, bv[:])

        ones_f = const.tile([P, 1], FP32)
        nc.vector.memset(ones_f, 1.0)
        ones16 = const.tile([P, 1], FP16)
        nc.vector.tensor_copy(ones16, ones_f)
        one1f = const.tile([1, 1], FP32)
        nc.vector.memset(one1f, 1.0)
        one116 = const.tile([1, 1], FP16)
        nc.vector.tensor_copy(one116, one1f)

        kvT = const.tile([P, NKV], F32R)  # [c, m]
        qTin = const.tile([P, NQ], F32R)  # [c, n]
        kT = const.tile([P, NKV], F32R)  # [f, m]
        qT = const.tile([P, NQ], F32R)  # [f, n]
        vT = const.tile([P, NKV], F32R)  # [f, m]
        vt = const.tile([P, NKV_T, F], FP16)  # [m%128, m//128, f]

        def load_group(view, g, dstT, tagc, evict_engine):
            """DMA one [128, 512] interleaved group, transpose its 4 blocks
            into one PSUM tile, evict coalesced (casts to f32r)."""
            xt = _PRELOADED.get((tagc, g))
            if xt is None:
                xt = xpool.tile([P, TQ * C], FP32, tag="xt", name=f"x{tagc}_{g}")
                (nc.sync if g % 2 == 0 else nc.scalar).dma_start(xt, view[g])
            pt = pwork.tile([P, TQ * P], FP32, tag="work", name=f"p{tagc}_{g}")
            for t in range(TQ):
                nc.tensor.transpose(
                    pt[:, t * P : (t + 1) * P], xt[:, t * P : (t + 1) * P], identity
                )
            col = g * (P * TQ)
            if evict_engine == "act":
                nc.scalar.copy(dstT[:, col : col + TQ * P], pt)
            else:
                nc.vector.tensor_copy(dstT[:, col : col + TQ * P], pt)

        def project_slice(wname, srcT, dstT, bias, j):
            pp = pwork.tile([P, MM], FP32, tag="work", name=f"pj{wname}_{j}")
            nc.tensor.matmul(
                pp, w_s[wname], srcT[:, j * MM : (j + 1) * MM], start=True, stop=True
            )
            nc.vector.tensor_scalar_add(dstT[:, j * MM : (j + 1) * MM], pp, bias)

        def vt_group(g):
            """Transpose 4 vT blocks into fp16 vt tiles (one coalesced evict)."""
            pv = pwork.tile([P, TQ * P], F32R, tag="work", name=f"pvt_{g}")
            for t in range(TQ):
                i = g * TQ + t
                nc.tensor.transpose(
                    pv[:, t * P : (t + 1) * P],
                    vT[:, i * P : (i + 1) * P],
                    identity_r,
                )
            nc.vector.tensor_copy(vt[:, g * TQ : (g + 1) * TQ, :], pv)

        # ---- queries for chunk 0 (rest interleaved below) ----
        for g in range(2):
            load_group(xq_v, g, qTin, "q", "dve")
        for j in range(2):
            project_slice("wq", qTin, qT, bq_s, j)

        # ---- attention chunk emitter (lagged PV, fp16 denominator) ----
        chunk_state = {}

        def attn_start(nch):
            oT = opsum.tile([P, NCHUNK], FP32, tag="oT", name=f"oT_{nch}")
            acc = apool.tile([P, NCHUNK], FP16, tag="acc", name=f"acc_{nch}")
            chunk_state[nch] = dict(oT=oT, acc=acc, e0=None, eq=[])

        def emit_pv(nch, e, mi):
            st = chunk_state[nch]
            for h in range(NCHUNK // MM):
                nc.tensor.matmul(
                    st["oT"][:, h * MM : (h + 1) * MM],
                    vt[:, mi, :],
                    e[:, h * MM : (h + 1) * MM],
                    start=(mi == 0),
                    stop=(mi == NKV_T - 1),
                )

        def attn_mi(nch, mi, lag=1):
            st = chunk_state[nch]
            nq0 = nch * NCHUNK
            sp = spsum.tile([P, NCHUNK], FP32, tag="sp", name=f"sp_{nch}_{mi}")
            for h in range(NCHUNK // MM):
                nc.tensor.matmul(
                    sp[:, h * MM : (h + 1) * MM],
                    kT[:, mi * P : (mi + 1) * P],
                    qT[:, nq0 + h * MM : nq0 + (h + 1) * MM],
                    start=True,
                    stop=True,
                )
            e = epool.tile([P, NCHUNK], FP16, tag="e", name=f"e_{nch}_{mi}")
            nc.scalar.activation(e, sp, EXP, scale=SCALE)
            # fp16 denominator partial sums on DVE; the last tile skips its
            # add (attn_finish folds it straight into dn).
            if mi == 0:
                st["e0"] = e
            elif mi == 1:
                nc.vector.tensor_tensor(st["acc"], st["e0"], e, ADD)
            elif mi < NKV_T - 1:
                nc.vector.tensor_tensor(st["acc"], st["acc"], e, ADD)
            st["eq"].append((e, mi))
            if len(st["eq"]) > lag:
                emit_pv(nch, *st["eq"].pop(0))

        def attn_finish(nch, osb_engine):
            st = chunk_state[nch]
            e_last = st["eq"][-1][0]
            while st["eq"]:
                emit_pv(nch, *st["eq"].pop(0))
            # partition-fold per 512-half: dn[1, n] = sum_m (acc + e_last)
            rdns = []
            for h in range(NCHUNK // MM):
                hs = slice(h * MM, (h + 1) * MM)
                dn = pwork.tile([1, MM], FP32, tag="work", name=f"dn_{nch}_{h}")
                nc.tensor.matmul(dn, ones16, st["acc"][:, hs], start=True, stop=False)
                nc.tensor.matmul(dn, ones16, e_last[:, hs], start=False, stop=True)
                rdn = npool.tile([1, MM], FP32, tag="rdn", name=f"rdn_{nch}_{h}")
                nc.vector.reciprocal_approx_fast(rdn, dn)
                rdn16 = npool.tile([1, MM], FP16, tag="rdn16", name=f"rdn16_{nch}_{h}")
                nc.vector.tensor_copy(rdn16, rdn)
                rdns.append(rdn16)
            # transpose 1/d rows into per-column scalars dcol[128, j] via
            # tiny single-pass fp16 matmuls (fp32 would emit LOW+HIGH pairs)
            dcol = pwork.tile([P, NCHUNK // P], FP32, tag="work", name=f"dcol_{nch}")
            for j in range(NCHUNK // P):
                rdn = rdns[j // TQ]
                jj = j % TQ
                nc.tensor.matmul(
                    dcol[:, j : j + 1],
                    rdn[0:1, jj * P : (jj + 1) * P],
                    one116,
                    start=True,
                    stop=True,
                )
            dcol_sb = npool.tile([P, NCHUNK // P], FP32, tag="dcol", name=f"dcs_{nch}")
            nc.vector.tensor_copy(dcol_sb, dcol)
            # unnormalized eviction of oT (runs in parallel with the dn chain)
            osb = npool.tile([P, NCHUNK], FP32, tag="osb", name=f"osb_{nch}")
            if osb_engine == "act":
                nc.scalar.copy(osb, st["oT"])
            else:
                nc.vector.tensor_copy(osb, st["oT"])
            for gg in range(NCHUNK // (P * TQ)):
                g = nch * (NCHUNK // (P * TQ)) + gg
                tp = pwork.tile([P, TQ * P], FP32, tag="work", name=f"tp_{nch}_{gg}")
                for t in range(TQ):
                    j = gg * TQ + t
                    nc.tensor.transpose(
                        tp[:, t * P : (t + 1) * P], osb[:, j * P : (j + 1) * P], identity
                    )
                ot = otpool.tile([P, TQ * P], FP32, tag="ot", name=f"ot_{nch}_{gg}")
                for t in range(TQ):
                    j = gg * TQ + t
                    if t % 2 == 0:
                        nc.vector.tensor_scalar_mul(
                            ot[:, t * P : (t + 1) * P],
                            tp[:, t * P : (t + 1) * P],
                            dcol_sb[:, j : j + 1],
                        )
                    else:
                        nc.scalar.activation(
                            ot[:, t * P : (t + 1) * P],
                            tp[:, t * P : (t + 1) * P],
                            CPY,
                            scale=dcol_sb[:, j : j + 1],
                        )
                (nc.sync if gg % 2 == 0 else nc.scalar).dma_start(
                    out_v[g], ot.rearrange("p (t c) -> p t c", t=TQ)
                )

        # ---- chunk 0, kv group g+1 loaded/projected ahead of group g's
        # ---- attention so PSUM evictions never gate the PE
        attn_start(0)
        load_group(xkv_v, 0, kvT, "k", "act")
        project_slice("wk", kvT, kT, bk_s, 0)
        project_slice("wv", kvT, vT, bv_s, 0)
        for g in range(NGK):
            if g + 1 < NGK:
                load_group(xkv_v, g + 1, kvT, "k", "act")
                project_slice("wk", kvT, kT, bk_s, g + 1)
                project_slice("wv", kvT, vT, bv_s, g + 1)
            vt_group(g)
            if g < 2:  # finish the q-side for chunk 1
                load_group(xq_v, g + 2, qTin, "q", "dve")
                project_slice("wq", qTin, qT, bq_s, g + 2)
            for t in range(TQ):
                attn_mi(0, g * TQ + t, lag=1)

        # chunk 1 head first (lag-2 PVs bridge the single-buffered oT drain),
        # then chunk-0 tail, then the rest
        attn_start(1)
        for mi in range(3):
            attn_mi(1, mi, lag=2)
        attn_finish(0, osb_engine="dve")
        for mi in range(3, NKV_T):
            attn_mi(1, mi, lag=2)
        attn_finish(1, osb_engine="act")

    nc.compile()
    return nc


def _get_nc():
    if "nc" not in _CACHE:
        _CACHE["nc"] = _build_nc()
    return _CACHE["nc"]


def run(inputs, trace=False, **kwargs):
    """Run on 8 cores; returns (full_output [4,4096,128], BassKernelResults)."""
    from concourse.bass_utils import run_bass_kernel_spmd

    q_in = np.ascontiguousarray(np.asarray(inputs["q_inputs"], dtype=np.float32))
    kv_in = np.ascontiguousarray(np.asarray(inputs["kv_inputs"], dtype=np.float32))
    wq = np.ascontiguousarray(np.asarray(inputs["Wq"], dtype=np.float32))
    wk = np.ascontiguousarray(np.asarray(inputs["Wk"], dtype=np.float32))
    wv = np.ascontiguousarray(np.asarray(inputs["Wv"], dtype=np.float32))
    bq = np.ascontiguousarray(np.asarray(inputs["bq"], dtype=np.float32).reshape(F, 1))
    bk = np.ascontiguousarray(np.asarray(inputs["bk"], dtype=np.float32).reshape(F, 1))
    bv = np.ascontiguousarray(np.asarray(inputs["bv"], dtype=np.float32).reshape(F, 1))

    halves = NQ_FULL // NQ  # 2
    in_maps = []
    for core in range(N_CORES):
        b, h = core // halves, core % halves
        in_maps.append(
            {
                "xq": np.ascontiguousarray(q_in[b, h * NQ : (h + 1) * NQ]),
                "xkv": np.ascontiguousarray(kv_in[b]),
                "wq": wq,
                "wk": wk,
                "wv": wv,
                "bq": bq,
                "bk": bk,
                "bv": bv,
            }
        )

    nc = _get_nc()
    res = run_bass_kernel_spmd(
        nc, in_maps, core_ids=list(range(N_CORES)), trace=trace, **kwargs
    )

    full = np.empty((B_FULL, NQ_FULL, F), dtype=np.float32)
    for core in range(N_CORES):
        b, h = core // halves, core % halves
        full[b, h * NQ : (h + 1) * NQ] = res.results[core]["out"]
    return full, res


def kernel(**inputs):
    full, _ = run(inputs, trace=False)
    return full
